# revision 7
# baseline (speedup 1.0000x reference)
"""MultiHeadAttention forward (nn.MultiheadAttention equivalent) on 8 trn2 NeuronCores.

Strategy: pure data parallelism over the batch dim (16 batches -> 2 per core, no
collectives). Per core, for each of its 2 batches:
  - Q/K/V projections as PE matmuls in float32r (full rate at N=512), consuming
    host-transposed activations (xT: [embed, tokens]) and host-transposed weights
    (W.T: [in, out]); the 1/sqrt(d) scale is folded into Wq on the host.
  - Scores computed in BOTH orientations straight from qT/kT slices (per head,
    d=64 on partitions):
      natural  s[n, m]  -> masked via a DVE add of a broadcast additive-mask
                           tile, exp on ACT with accum_out row sums, normalized
                           by the reciprocal sums (tensor_scalar) -> this is the
                           attn_weights output (DMA'd out m-contiguous).
      swapped  sT[m, n] -> masked for free via the per-partition exp bias
                           (mask indexed by m = partitions), giving pT, the
                           stationary-side operand for attention@V with no
                           on-chip transposes.
  - attention@V and the softmax denominators (ones-matmul column sums) for the
    pT path are computed as column-tiled matmul pairs (two heads concurrently in
    the PE array via tile_position), normalized on DVE, and assembled into
    attnT [e, n] which feeds the out-projection directly as the stationary
    operand. Normalization by softmax sums commutes with attention@V.
"""

import sys

sys.path.insert(0, "/opt/trn_rl_repo")

import numpy as np

import concourse.bass as bass
import concourse.mybir as mybir
import concourse.tile as tile
from concourse.bass_utils import run_bass_kernel_spmd

F32 = mybir.dt.float32
F32R = mybir.dt.float32r
AF = mybir.ActivationFunctionType

N_Q, N_KV, BATCH, EMBED, HEADS, D = 512, 512, 16, 1024, 16, 64
NCORES = 8
BPC = BATCH // NCORES  # batches per core
EC = EMBED // 128      # 8 embed chunks
QC = N_Q // 128        # 4 query-token chunks
KC = N_KV // 128       # 4 key-token chunks
NEG = -1.0e9
_ONES = np.ones((128, 128), dtype=np.float32)

LAST_RESULTS = None  # set by kernel(); holds BassKernelResults for profiling


def _split_multiwaits(nc, maxw=1):
    """This container's walrus rejects >1 sync wait on an instruction; move
    excess waits onto preceding same-engine no-ops."""
    for fn_ in nc.m.functions:
        for blk in fn_.blocks:
            insts = list(blk.instructions)
            for idx in range(len(insts) - 1, -1, -1):
                inst = insts[idx]
                si = inst.sync_info
                if si is None or len(si.on_wait) <= maxw:
                    continue
                waits = list(si.on_wait)
                keep = waits[-maxw:]
                extra = waits[:-maxw]
                inst.sync_info = mybir.SyncInfo(
                    on_wait=keep, on_update=list(si.on_update)
                )
                nops = []
                while extra:
                    grp, extra = extra[:maxw], extra[maxw:]
                    nop = mybir.InstNoOp(
                        name=f"{inst.name}-wsplit-{len(nops)}",
                        sync_info=mybir.SyncInfo(on_wait=grp, on_update=[]),
                        bass_nofuse=True,
                        engine=inst.engine,
                    )
                    nc.register_instruction(nop)
                    nops.append(nop)
                for k, nop in enumerate(nops):
                    blk.instructions.insert(idx + k, nop)


def _bcast_ap(ap_1d, parts):
    """Partition-broadcast (stride 0) DMA source from a 1-D DRAM AP."""
    return bass.AP(
        tensor=ap_1d.tensor, offset=ap_1d.offset, ap=[[0, parts]] + list(ap_1d.ap)
    )


def _build(has_bq, has_bk, has_bv, has_bo):
    nc = bass.Bass()

    xq = nc.declare_dram_parameter("xq", [BPC, EMBED, N_Q], F32R, isOutput=False)
    xk = nc.declare_dram_parameter("xk", [BPC, EMBED, N_KV], F32R, isOutput=False)
    xv = nc.declare_dram_parameter("xv", [BPC, EMBED, N_KV], F32R, isOutput=False)
    wq = nc.declare_dram_parameter("wq", [EMBED, EMBED], F32R, isOutput=False)
    wk = nc.declare_dram_parameter("wk", [EMBED, EMBED], F32R, isOutput=False)
    wv = nc.declare_dram_parameter("wv", [EMBED, EMBED], F32R, isOutput=False)
    wo = nc.declare_dram_parameter("wo", [EMBED, EMBED], F32R, isOutput=False)
    mk = nc.declare_dram_parameter("mk", [BPC, N_KV], F32, isOutput=False)
    onesd = nc.declare_dram_parameter("onesd", [128, 128], F32R, isOutput=False)
    if has_bq:
        bq = nc.declare_dram_parameter("bq", [EMBED], F32, isOutput=False)
    if has_bk:
        bk = nc.declare_dram_parameter("bk", [EMBED], F32, isOutput=False)
    if has_bv:
        bv = nc.declare_dram_parameter("bv", [EMBED], F32R, isOutput=False)
    if has_bo:
        bo = nc.declare_dram_parameter("bo", [EMBED], F32R, isOutput=False)
    out_d = nc.declare_dram_parameter("out", [N_Q, BPC, EMBED], F32, isOutput=True)
    wts_d = nc.declare_dram_parameter(
        "wts", [BPC, HEADS, N_Q, N_KV], F32, isOutput=True
    )

    def r(ap):
        return ap

    with tile.TileContext(nc) as tc:
        with (
            tc.tile_pool(name="qk", bufs=1) as qkp,       # qT/kT per batch
            tc.tile_pool(name="vp", bufs=1) as vp,        # v natural per batch
            tc.tile_pool(name="at", bufs=1) as atp,       # attnT per batch
            tc.tile_pool(name="wp", bufs=2) as wp,        # weight halves
            tc.tile_pool(name="mi", bufs=1) as mip,       # constants / masks
            tc.tile_pool(name="ps", bufs=8, space="PSUM") as ps,
        ):
            # ---- constants / masks ----
            onesT = mip.tile([128, 128], F32R)
            nc.sync.dma_start(out=onesT, in_=onesd[:, :])
            ones64 = onesT[:, 0:64]
            ones1 = onesT[0:1, :]
            maskb = []
            maskc = []
            for b in range(BPC):
                mb = mip.tile([128, N_KV], F32, tag=f"maskb{b}")
                nc.sync.dma_start(out=mb, in_=_bcast_ap(mk[b], 128))
                maskb.append(mb)
                mc_t = mip.tile([128, KC], F32, tag=f"maskc{b}")
                nc.sync.dma_start(
                    out=mc_t, in_=mk[b].rearrange("(c p) -> p c", p=128)
                )
                maskc.append(mc_t)
            if has_bq:
                bq_sb = mip.tile([128, EC], F32, tag="bq")
                nc.sync.dma_start(out=bq_sb, in_=bq.rearrange("(c p) -> p c", p=128))
            if has_bk:
                bk_sb = mip.tile([128, EC], F32, tag="bk")
                nc.sync.dma_start(out=bk_sb, in_=bk.rearrange("(c p) -> p c", p=128))
            if has_bv:
                bv_sb = mip.tile([1, EMBED], F32R, tag="bv")
                nc.sync.dma_start(out=bv_sb, in_=bv[None, :])
            if has_bo:
                bo_sb = mip.tile([1, EMBED], F32R, tag="bo")
                nc.sync.dma_start(out=bo_sb, in_=bo[None, :])

            qT = [qkp.tile([128, EC, N_Q], F32R, tag=f"q{b}", name=f"qT{b}") for b in range(BPC)]
            kT = [qkp.tile([128, EC, N_KV], F32R, tag=f"k{b}", name=f"kT{b}") for b in range(BPC)]
            vN = [vp.tile([128, KC, EMBED], F32R, tag=f"v{b}", name=f"vN{b}") for b in range(BPC)]

            # ---- projections ----
            with tc.tile_pool(name="xp", bufs=1) as xp:
                # q and k projections: qT[e, t] = sum_c W.T[c, e] * xT[c, t]
                for (xd, wd, dst, bias_sb) in (
                    (xq, wq, qT, bq_sb if has_bq else None),
                    (xk, wk, kT, bk_sb if has_bk else None),
                ):
                    xt = [xp.tile([128, EC, 512], F32R, name=f"xt{b}") for b in range(BPC)]
                    for b in range(BPC):
                        nc.sync.dma_start(
                            out=xt[b], in_=xd[b].rearrange("(c p) t -> p c t", p=128)
                        )
                    for half in range(2):
                        wt = wp.tile([128, EC, 512], F32R)
                        nc.sync.dma_start(
                            out=wt,
                            in_=wd.rearrange("(c p) e -> p c e", p=128)[
                                :, :, half * 512 : (half + 1) * 512
                            ],
                        )
                        for b in range(BPC):
                            for el in range(4):
                                ec = half * 4 + el
                                pt = ps.tile([128, 512], F32, tag="ps", name="pt")
                                for cc in range(EC):
                                    nc.tensor.matmul(
                                        pt,
                                        r(wt[:, cc, el * 128 : (el + 1) * 128]),
                                        r(xt[b][:, cc, :]),
                                        start=(cc == 0),
                                        stop=(cc == EC - 1),
                                    )
                                if bias_sb is not None:
                                    nc.scalar.activation(
                                        dst[b][:, ec, :], pt, AF.Identity,
                                        bias=bias_sb[:, ec : ec + 1],
                                    )
                                else:
                                    nc.scalar.copy(dst[b][:, ec, :], pt)

                # v projection (natural layout): v[t, e] = sum_c xT[c, t] * W.T[c, e]
                xt = [xp.tile([128, EC, 512], F32R, name=f"xt{b}") for b in range(BPC)]
                for b in range(BPC):
                    nc.sync.dma_start(
                        out=xt[b], in_=xv[b].rearrange("(c p) t -> p c t", p=128)
                    )
                for half in range(2):
                    wt = wp.tile([128, EC, 512], F32R)
                    nc.sync.dma_start(
                        out=wt,
                        in_=wv.rearrange("(c p) e -> p c e", p=128)[
                            :, :, half * 512 : (half + 1) * 512
                        ],
                    )
                    for b in range(BPC):
                        for tch in range(KC):
                            pt = ps.tile([128, 512], F32, tag="ps", name="pt")
                            for cc in range(EC):
                                nc.tensor.matmul(
                                    pt,
                                    r(xt[b][:, cc, tch * 128 : (tch + 1) * 128]),
                                    r(wt[:, cc, :]),
                                    start=(cc == 0),
                                    stop=(cc == EC - 1 and not has_bv),
                                )
                            if has_bv:
                                nc.tensor.matmul(
                                    pt,
                                    r(ones1[:, 0:128]),
                                    r(bv_sb[:, half * 512 : (half + 1) * 512]),
                                    start=False,
                                    stop=True,
                                )
                            nc.scalar.copy(
                                vN[b][:, tch, half * 512 : (half + 1) * 512], pt
                            )

            # out-projection weight halves (loaded once, used for both batches)
            wo_t = []
            for half in range(2):
                wt = wp.tile([128, EC, 512], F32R)
                nc.sync.dma_start(
                    out=wt,
                    in_=wo.rearrange("(c p) e -> p c e", p=128)[
                        :, :, half * 512 : (half + 1) * 512
                    ],
                )
                wo_t.append(wt)

            # ---- attention + out-projection, per batch ----
            with (
                tc.tile_pool(name="pt", bufs=2) as ptp,
                tc.tile_pool(name="pn", bufs=5) as pnp,
                tc.tile_pool(name="rr", bufs=4) as rrp,
                tc.tile_pool(name="rc", bufs=2) as rcp,
                tc.tile_pool(name="ob", bufs=2) as obp,
            ):
                for b in range(BPC):
                    atT = atp.tile([128, EC, N_Q], F32R)
                    for j in range(HEADS // 2):
                        h0, h1 = 2 * j, 2 * j + 1
                        pT = ptp.tile([128, KC, 2, N_Q], F32R)
                        for hh, h in ((0, h0), (1, h1)):
                            ro = 64 * (h % 2)
                            # swapped scores sT[m, n]; mask via per-partition bias
                            for mc in range(KC):
                                st = ps.tile([128, 512], F32, tag="ps", name="st")
                                nc.tensor.matmul(
                                    st,
                                    r(kT[b][ro : ro + 64, j, mc * 128 : (mc + 1) * 128]),
                                    r(qT[b][ro : ro + 64, j, :]),
                                )
                                nc.scalar.activation(
                                    pT[:, mc, hh, :], st, AF.Exp,
                                    bias=maskc[b][:, mc : mc + 1],
                                )
                            # natural scores s[n, m] -> attn_weights output
                            rs = rrp.tile([128, QC], F32)
                            pn_tiles = []
                            for qc_ in range(QC):
                                sn = ps.tile([128, 512], F32, tag="ps", name="sn")
                                nc.tensor.matmul(
                                    sn,
                                    r(qT[b][ro : ro + 64, j, qc_ * 128 : (qc_ + 1) * 128]),
                                    r(kT[b][ro : ro + 64, j, :]),
                                )
                                nc.vector.tensor_add(sn, sn, maskb[b])
                                pn = pnp.tile([128, 512], F32)
                                nc.scalar.activation(
                                    pn, sn, AF.Exp, accum_out=rs[:, qc_ : qc_ + 1]
                                )
                                pn_tiles.append(pn)
                            rsr = rrp.tile([128, QC], F32)
                            nc.vector.reciprocal(rsr, rs)
                            for qc_ in range(QC):
                                nc.vector.tensor_scalar_mul(
                                    pn_tiles[qc_], pn_tiles[qc_], rsr[:, qc_ : qc_ + 1]
                                )
                                nc.sync.dma_start(
                                    out=wts_d[b, h, qc_ * 128 : (qc_ + 1) * 128, :],
                                    in_=pn_tiles[qc_],
                                )
                        # attention @ V + denominator sums, per head (M=64)
                        for hh, h in ((0, h0), (1, h1)):
                            av = ps.tile([128, 512], F32, tag="ps", name="av")
                            for mc in range(KC):
                                nc.tensor.matmul(
                                    av[0:64, :],
                                    r(vN[b][:, mc, h * 64 : h * 64 + 64]),
                                    r(pT[:, mc, hh, :]),
                                    start=(mc == 0),
                                    stop=(mc == KC - 1),
                                )
                            rb = ps.tile([128, 512], F32, tag="ps", name="rb")
                            for mc in range(KC):
                                nc.tensor.matmul(
                                    rb[0:64, :],
                                    r(ones64),
                                    r(pT[:, mc, hh, :]),
                                    start=(mc == 0),
                                    stop=(mc == KC - 1),
                                )
                            rc_t = rcp.tile([128, 512], F32, name="rc_t")
                            nc.vector.reciprocal(rc_t[0:64, :], rb[0:64, :])
                            if hh == 0:
                                nc.vector.tensor_mul(
                                    atT[0:64, j, :], av[0:64, :], rc_t[0:64, :]
                                )
                            else:
                                ah = rcp.tile([128, 512], F32R, name="ah")
                                nc.vector.tensor_mul(
                                    ah[0:64, :], av[0:64, :], rc_t[0:64, :]
                                )
                                nc.sync.dma_start(
                                    out=atT[64:128, j, :], in_=ah[0:64, :]
                                )

                    # out projection for batch b
                    for nq in range(QC):
                        ot = obp.tile([128, EMBED], F32)
                        for half in range(2):
                            po = ps.tile([128, 512], F32, tag="ps", name="po")
                            for ec in range(EC):
                                nc.tensor.matmul(
                                    po,
                                    r(atT[:, ec, nq * 128 : (nq + 1) * 128]),
                                    r(wo_t[half][:, ec, :]),
                                    start=(ec == 0),
                                    stop=(ec == EC - 1 and not has_bo),
                                )
                            if has_bo:
                                nc.tensor.matmul(
                                    po,
                                    r(ones1[:, 0:128]),
                                    r(bo_sb[:, half * 512 : (half + 1) * 512]),
                                    start=False,
                                    stop=True,
                                )
                            nc.scalar.copy(ot[:, half * 512 : (half + 1) * 512], po)
                        nc.sync.dma_start(
                            out=out_d[nq * 128 : (nq + 1) * 128, b, :], in_=ot
                        )

    _split_multiwaits(nc, maxw=1)
    return nc


_PROGRAM_CACHE = {}


def prepare(query, key, value, key_padding_mask, Wq, bq, Wk, bk, Wv, bv, Wo, bo):
    """Build (nc, in_maps) for the SPMD run; used by kernel() and benchmarks."""
    query = np.ascontiguousarray(np.asarray(query, dtype=np.float32))
    key = np.ascontiguousarray(np.asarray(key, dtype=np.float32))
    value = np.ascontiguousarray(np.asarray(value, dtype=np.float32))
    key_padding_mask = np.asarray(key_padding_mask)
    Wq = np.asarray(Wq, dtype=np.float32)
    Wk = np.asarray(Wk, dtype=np.float32)
    Wv = np.asarray(Wv, dtype=np.float32)
    Wo = np.asarray(Wo, dtype=np.float32)
    bq = np.asarray(bq, dtype=np.float32)
    bk = np.asarray(bk, dtype=np.float32)
    bv = np.asarray(bv, dtype=np.float32)
    bo = np.asarray(bo, dtype=np.float32)

    has_b = tuple(bool(np.any(x)) for x in (bq, bk, bv, bo))
    if has_b not in _PROGRAM_CACHE:
        _PROGRAM_CACHE[has_b] = _build(*has_b)
    nc = _PROGRAM_CACHE[has_b]

    scale = 1.0 / np.sqrt(np.float32(D))
    wq_h = np.ascontiguousarray(Wq.T * scale)   # [in, out], q-scale folded
    wk_h = np.ascontiguousarray(Wk.T)
    wv_h = np.ascontiguousarray(Wv.T)
    wo_h = np.ascontiguousarray(Wo.T)
    bq_h = bq * scale
    mask_add = np.where(key_padding_mask, np.float32(NEG), np.float32(0.0)).astype(
        np.float32
    )  # [BATCH, N_KV]

    in_maps = []
    for c in range(NCORES):
        bsel = range(c * BPC, (c + 1) * BPC)
        m = {
            "xq": np.ascontiguousarray(
                np.stack([query[:, b, :].T for b in bsel])
            ),
            "xk": np.ascontiguousarray(np.stack([key[:, b, :].T for b in bsel])),
            "xv": np.ascontiguousarray(np.stack([value[:, b, :].T for b in bsel])),
            "wq": wq_h, "wk": wk_h, "wv": wv_h, "wo": wo_h,
            "mk": np.ascontiguousarray(mask_add[list(bsel)]),
            "onesd": _ONES,
        }
        if has_b[0]:
            m["bq"] = bq_h
        if has_b[1]:
            m["bk"] = bk
        if has_b[2]:
            m["bv"] = bv
        if has_b[3]:
            m["bo"] = bo
        in_maps.append(m)
    return nc, in_maps


def kernel(**inputs):
    global LAST_RESULTS
    nc, in_maps = prepare(**inputs)
    res = run_bass_kernel_spmd(nc, in_maps, core_ids=list(range(NCORES)))
    LAST_RESULTS = res

    attn_output = np.concatenate(
        [res.results[c]["out"] for c in range(NCORES)], axis=1
    )  # [N_Q, BATCH, EMBED]
    attn_weights = np.concatenate(
        [res.results[c]["wts"] for c in range(NCORES)], axis=0
    )  # [BATCH, HEADS, N_Q, N_KV]
    return attn_output, attn_weights
